# revision 21
# baseline (speedup 1.0000x reference)
"""Trainium2 Bass kernel: attention layer (B=4, S=2048, D=1024), 8 NeuronCores.

Sharding: data-parallel over (batch, query-half) -> 8 shards. Each core
computes one batch's half of the queries against that batch's full keys.

Algorithm (reassociated to cut PE work ~22% vs the direct form):
  scores = (q Wq^T)(key Wk^T)^T = q (Wq^T Wk) key^T
    -> MT = Wq^T Wk (128 matmuls, needs only the two weight matrices so
       the PE starts ~2 tile-loads into the kernel), AT = MT qT (128),
       then scores contract the RAW key (kT input, f32r) with AT (256)
       instead of Q-proj(128) + K-proj(256) + scores(256).
  out = P (key Wv^T) = (P key) Wv^T
    -> contract P with the raw key first (U^T = key^T E, 256 matmuls),
       then one 1024x1024 transform (128 matmuls) instead of projecting
       V for all 2048 keys (256) + PV (256). V is never materialized.
This removes every matmul that was redundant between the two cores of a
batch pair, with zero communication. ~898 matmuls/core vs 1155.

Per-core dataflow:
  P0  MT[d,d'] = Wq^T Wk      (f32r; e-inner sweeps of 4 d-groups pace
                               the PE at the wq/wk DMA arrival rate)
  P1  AT[d',q] = MT qT        (f32r; qT arrives during P0)
  P2  ST[k,q]  = kT^T AT      (f32r; k on partitions; row-max tracked by
                               DVE max chain; first 8 k-tiles spill to
                               DRAM, 8 stay resident; after the k=7
                               chain the kraw/wv loads and ST reloads
                               are issued so every DMA lands before the
                               softmax window)
  P3  m_bc = gpsimd.partition_all_reduce(max) over macc -- replaces the
      7-step DMA-shift halving + DRAM bounce broadcast of the baseline
      (whose serial semaphore+DMA latency cost ~40us of PE idle)
  P4  E = exp(ST - m) in bf16, qh-half-major; l accumulated on DVE
  P5  UT[d,q]  = key^T E      (bf16; lhsT = raw bf16 key tiles; 4
                               concurrent PSUM groups, k-outer, so PE
                               consumption paces the exp chain)
  P6  O[q,e]   = UT^T Wv^T    (bf16; the l ones-matmuls, PE-transpose of
                               1/l via DRAM bounce, and scaled stores
                               threaded into the O loop)

SBUF is managed as flat always-open pools of uniform 4KB/partition
slots with tag-rotation reuse (victim's last read always precedes the
new tile's first write):
  A(64KB,16): kT halves     -> kraw bf16 pairs (8) -> E pairs (8)
  B(32KB, 8): Wq tiles      -> m_bc,macc,lacc,(spare) -> O out-stage
  C(32KB, 8): Wk tiles      -> AT -> wv bf16 pairs (4) + UT pairs (4)
  D(32KB, 8): MT            -> ST residents (k=8..15)
  Q(32KB, 8): qT tiles      -> ST reloads (k=0..7)
Total ~198KB of ~203 usable. float32r keeps ~1.5e-4 relative precision
on the unscaled (logit std ~34) softmax path; bf16 is fine for E and
the U/O contractions.
"""

import numpy as np
import ml_dtypes
from contextlib import ExitStack

import concourse.bass as bass
import concourse.tile as tile
from concourse import bacc, bass_isa, mybir
from concourse.bass import ts
from concourse.bass_utils import run_bass_kernel_spmd

B, S, D = 4, 2048, 1024
N_CORES = 8
SQ = S // 2            # 1024 query rows per core
P = 128                # partitions
NE = D // P            # 8 e-tiles
ND = D // P            # 8 d-tiles
NK = S // P            # 16 k-tiles
NQC = SQ // P          # 8 q-chunks
NSPILL = 8             # ST k-tiles spilled to DRAM (rest stay resident)
F32R = mybir.dt.float32r
F32 = mybir.dt.float32
BF16 = mybir.dt.bfloat16

# E/UT production+consumption order: resident ST tiles (k=8..15) first.
KORDER = list(range(NSPILL, NK)) + list(range(NSPILL))

_NC_CACHE = {}


def _build():
    if "nc" in _NC_CACHE:
        return _NC_CACHE["nc"]
    nc = bacc.Bacc("TRN2", target_bir_lowering=False, debug=False,
                   num_devices=N_CORES)

    qT = nc.dram_tensor("qT", [D, SQ], F32R, kind="ExternalInput")
    kT = nc.dram_tensor("kT", [D, S], F32R, kind="ExternalInput")
    kraw = nc.dram_tensor("kraw", [S, D], BF16, kind="ExternalInput")
    wq = nc.dram_tensor("wq", [D, D], F32R, kind="ExternalInput")
    wk = nc.dram_tensor("wk", [D, D], F32R, kind="ExternalInput")
    wvT = nc.dram_tensor("wvT", [D, D], BF16, kind="ExternalInput")
    out = nc.dram_tensor("out", [SQ, D], F32, kind="ExternalOutput")

    from concourse.masks import make_identity

    with tile.TileContext(nc) as tc:
        with ExitStack() as ctx:
            psum = ctx.enter_context(tc.tile_pool(name="psum", bufs=6, space="PSUM"))
            psl = ctx.enter_context(tc.tile_pool(name="psl", bufs=1, space="PSUM"))
            dram = ctx.enter_context(tc.tile_pool(name="dram", bufs=1, space="DRAM"))
            consts = ctx.enter_context(tc.tile_pool(name="consts", bufs=1))
            tiny = ctx.enter_context(tc.tile_pool(name="tiny", bufs=1))
            pA = ctx.enter_context(tc.tile_pool(name="pA", bufs=16))
            pB = ctx.enter_context(tc.tile_pool(name="pB", bufs=8))
            pC = ctx.enter_context(tc.tile_pool(name="pC", bufs=8))
            pD = ctx.enter_context(tc.tile_pool(name="pD", bufs=8))
            pQ = ctx.enter_context(tc.tile_pool(name="pQ", bufs=8))

            id8 = consts.tile([8, 8], F32)
            make_identity(nc, id8[:])
            ones_c = consts.tile([P, 1], F32)
            nc.gpsimd.memset(ones_c[:], 1.0)

            st_spill = [dram.tile([P, SQ], F32, tag="stsp", name=f"stsp{i}")
                        for i in range(NSPILL)]

            dmae = [nc.sync, nc.scalar, nc.gpsimd]

            # ---- tile allocation in rotation order -----------------
            # wq/wk live as separate lo/hi column-half tiles (packed 2
            # per 4KB slot): wave-2 DMA writes then land in different
            # slots than the halves the P0 sweeps are reading, avoiding
            # SBUF read/write bank conflicts (measured ~6us of 300-430ns
            # matmuls when a single tile was half-read, half-written).
            wqlo = [pB.tile([P, D], F32R, tag="b", name=f"wqlo{j}")
                    for j in range(4)]
            wqhi = [pB.tile([P, D], F32R, tag="b", name=f"wqhi{j}")
                    for j in range(4)]
            wklo = [pC.tile([P, D], F32R, tag="c", name=f"wklo{j}")
                    for j in range(4)]
            wkhi = [pC.tile([P, D], F32R, tag="c", name=f"wkhi{j}")
                    for j in range(4)]

            def wq_ap(e, c0, w_):   # wq[e-tile][:, c0:c0+w_], c0 half-aligned
                src = wqlo if c0 < 512 else wqhi
                base = (e % 2) * 512 + (c0 % 512)
                return src[e // 2][:, base:base + w_]

            def wk_ap(e, c0, w_):
                src = wklo if c0 < 512 else wkhi
                base = (e % 2) * 512 + (c0 % 512)
                return src[e // 2][:, base:base + w_]
            qts = [pQ.tile([P, SQ], F32R, tag="q", name=f"qin{i}")
                   for i in range(ND)]
            ktsA = [pA.tile([P, SQ], F32R, tag="a", name=f"kta{i}")
                    for i in range(ND)]
            ktsB = [pA.tile([P, SQ], F32R, tag="a", name=f"ktb{i}")
                    for i in range(ND)]
            MT = [pD.tile([P, D], F32R, tag="d", name=f"mt{i}")
                  for i in range(ND)]

            # input loads in consumption order, round-robin 3 queues:
            # wq/wk pairs feed P0 immediately; qT lands during P0 for
            # P1; kT during P1 for P2. kraw/wv/reloads are issued later
            # (inside P2) once their victim slots are dead.
            for d in range(ND):
                dmae[(2 * d) % 3].dma_start(wq_ap(d, 0, 512),
                                            wq.ap()[ts(d, P), 0:512])
                dmae[(2 * d + 1) % 3].dma_start(wk_ap(d, 0, 512),
                                                wk.ap()[ts(d, P), 0:512])
            for d in range(ND):
                dmae[(2 * d) % 3].dma_start(wq_ap(d, 512, 512),
                                            wq.ap()[ts(d, P), 512:D])
                dmae[(2 * d + 1) % 3].dma_start(wk_ap(d, 512, 512),
                                                wk.ap()[ts(d, P), 512:D])
            for d in range(ND):
                dmae[d % 3].dma_start(qts[d][:], qT.ap()[ts(d, P), :])
            for d in range(ND):
                dmae[(2 * d) % 3].dma_start(ktsA[d][:], kT.ap()[ts(d, P), 0:SQ])
                dmae[(2 * d + 1) % 3].dma_start(ktsB[d][:], kT.ap()[ts(d, P), SQ:S])

            # ---- P0: MT[d,d'] = Wq^T Wk ----------------------------
            # e-inner over 4-d-groups: the first matmul needs only
            # wqs[0]+wks[0]; each arriving e-pair feeds 4 matmuls.
            for half in range(2):
                for dg in range(2):
                    pss = [psum.tile([P, 512], F32, tag="mm",
                                     name=f"ps_m{half}_{dg}_{i}")
                           for i in range(4)]
                    for e in range(NE):
                        for i in range(4):
                            nc.tensor.matmul(pss[i][:],
                                             wq_ap(e, (dg * 4 + i) * P, P),
                                             wk_ap(e, half * 512, 512),
                                             start=(e == 0),
                                             stop=(e == NE - 1))
                    for i in range(4):
                        nc.vector.tensor_copy(
                            MT[dg * 4 + i][:, ts(half, 512)], pss[i][:])

            # ---- P1: AT[d',q] = MT qT ------------------------------
            AT = [pC.tile([P, SQ], F32R, tag="c", name=f"at{i}")
                  for i in range(ND)]
            for qh in range(2):
                for dg in range(2):
                    pss = [psum.tile([P, 512], F32, tag="mm",
                                     name=f"ps_a{qh}_{dg}_{i}")
                           for i in range(4)]
                    for d in range(ND):
                        for i in range(4):
                            nc.tensor.matmul(pss[i][:],
                                             MT[d][:, ts(dg * 4 + i, P)],
                                             qts[d][:, ts(qh, 512)],
                                             start=(d == 0),
                                             stop=(d == ND - 1))
                    for i in range(4):
                        nc.vector.tensor_copy(
                            AT[dg * 4 + i][:, ts(qh, 512)], pss[i][:])

            # softmax scratch lands in B (wq victims, dead after P0)
            m_bc = pB.tile([P, SQ], F32, tag="b", name="m_bc")
            macc = pB.tile([P, SQ], F32, tag="b", name="macc")
            lacc = pB.tile([P, SQ], F32, tag="b", name="lacc")
            spare = pB.tile([P, SQ], F32, tag="b", name="spare")  # noqa: F841

            # ---- P2: ST[k,q] = kT^T AT; DVE row-max on the fly -----
            # k=0..7 spill to DRAM (slot reused 8 tiles later by the
            # rotation); k=8..15 stay resident in D (MT victims).
            st_tiles = {}
            krs2 = []

            def st_chain(st_k, k, qh):
                # one (k, qh) score chain + drain + per-half running
                # max; each half's partition reduce launches the moment
                # the last chain for that half drains. The final two k
                # iterations are emitted half-interleaved (qh0 chains
                # of k=14,15 first) so the qh0 reduce hides entirely
                # under the remaining qh1 chains.
                sl = ts(qh, 512)
                kts = ktsA if k < 8 else ktsB
                ps = psum.tile([P, 512], F32, tag="mm", name=f"ps_s{k}_{qh}")
                for dp in range(ND):
                    nc.tensor.matmul(ps[:], kts[dp][:, ts(k % 8, P)],
                                     AT[dp][:, ts(qh, 512)],
                                     start=(dp == 0), stop=(dp == ND - 1))
                nc.vector.tensor_copy(st_k[:, sl], ps[:])
                if k == 0:
                    nc.vector.tensor_copy(macc[:, sl], st_k[:, sl])
                else:
                    nc.vector.tensor_max(macc[:, sl], macc[:, sl],
                                         st_k[:, sl])
                if k == NK - 1:
                    nc.gpsimd.partition_all_reduce(
                        m_bc[:, sl], macc[:, sl], channels=P,
                        reduce_op=bass_isa.ReduceOp.max)

            for k in range(NK - 2):
                st_k = pD.tile([P, SQ], F32, tag="d", name=f"stb{k}")
                for qh in range(2):
                    st_chain(st_k, k, qh)
                if k < NSPILL:
                    nc.sync.dma_start(st_spill[k][:], st_k[:])
                else:
                    st_tiles[k] = st_k
                if k == 7:
                    # ktsA is dead: issue kraw (its victim) now so the
                    # 4MB lands under P2's second half; ST reloads (qts
                    # victims, dead since P1) prefetch on gpsimd.
                    krs2 = [pA.tile([P, 2 * SQ], BF16, tag="a",
                                    name=f"kr{j}") for j in range(NK // 2)]
                    for j in range(NK // 2):
                        for h in range(2):
                            kk2 = KORDER[2 * j + h]
                            dmae[(2 * j + h) % 2].dma_start(
                                krs2[j][:, ts(h, SQ)],
                                kraw.ap()[ts(kk2, P), :])
                    for kr in range(NSPILL):
                        st_r = pQ.tile([P, SQ], F32, tag="q",
                                       name=f"rl{kr}")
                        nc.gpsimd.dma_start(st_r[:], st_spill[kr][:])
                        st_tiles[kr] = st_r
            st14 = pD.tile([P, SQ], F32, tag="d", name="stb14")
            st15 = pD.tile([P, SQ], F32, tag="d", name="stb15")
            st_tiles[NK - 2], st_tiles[NK - 1] = st14, st15
            st_chain(st14, NK - 2, 0)
            st_chain(st15, NK - 1, 0)
            st_chain(st14, NK - 2, 1)
            st_chain(st15, NK - 1, 1)

            # wv (AT victims, dead at P2 end) + UT slots in C
            wv2 = [pC.tile([P, 2 * SQ], BF16, tag="c", name=f"wv{j}")
                   for j in range(ND // 2)]
            for j in range(ND // 2):
                for h in range(2):
                    nc.sync.dma_start(
                        wv2[j][:, ts(h, SQ)], wvT.ap()[ts(2 * j + h, P), :])
            ut2 = [pC.tile([P, 2 * SQ], BF16, tag="c", name=f"ut{j}")
                   for j in range(ND // 2)]

            # ---- P4+P5 interleaved: E = exp(ST - m) bf16, l on DVE;
            # UT[d,q] = key^T E, 4 concurrent PSUM groups, k-outer ----
            E2 = [pA.tile([P, 2 * SQ], BF16, tag="a", name=f"e{j}")
                  for j in range(NK // 2)]

            def e_ap(i, qh):        # [P,512] E view for KORDER[i]
                return E2[i // 2][:, (i % 2) * SQ + qh * 512:
                                  (i % 2) * SQ + qh * 512 + 512]

            def kr_ap(i, dp):       # lhsT slice for KORDER[i], d-tile dp
                return krs2[i // 2][:, (i % 2) * SQ + dp * P:
                                    (i % 2) * SQ + (dp + 1) * P]

            def ut_ap(dp, c0, w_):  # [P,w_] UT view for d-tile dp
                return ut2[dp // 2][:, (dp % 2) * SQ + c0:
                                    (dp % 2) * SQ + c0 + w_]

            for qh in range(2):
                sl = ts(qh, 512)
                for i, k in enumerate(KORDER):
                    st_k = st_tiles[k]
                    nc.vector.tensor_sub(st_k[:, sl], st_k[:, sl],
                                         m_bc[:, sl])
                    nc.scalar.activation(e_ap(i, qh), st_k[:, sl],
                                         mybir.ActivationFunctionType.Exp)
                    if i == 1:
                        nc.vector.tensor_add(lacc[:, sl], e_ap(0, qh),
                                             e_ap(1, qh))
                    elif i > 1:
                        nc.vector.tensor_add(lacc[:, sl], lacc[:, sl],
                                             e_ap(i, qh))
                # 6-group then 2-group sweeps: the wider first sweep
                # consumes each E tile 6x (1.4us) vs the 0.7us exp
                # cadence, so the PE rides out the exp-chain warmup.
                for d0, gw in ((0, 6), (6, 2)):
                    pss = [psum.tile([P, 512], F32, tag="mm",
                                     name=f"ps_u{qh}_{d0}_{i}")
                           for i in range(gw)]
                    for i in range(NK):
                        for t in range(gw):
                            nc.tensor.matmul(pss[t][:],
                                             kr_ap(i, d0 + t),
                                             e_ap(i, qh),
                                             start=(i == 0),
                                             stop=(i == NK - 1))
                    for t in range(gw):
                        nc.vector.tensor_copy(
                            ut_ap(d0 + t, qh * 512, 512), pss[t][:])

            # ---- P6: O[q,e] = UT^T Wv^T; 1/l path threaded in ------
            groups = [(qc, eh) for qc in range(NQC) for eh in range(D // 512)]
            l_row = tiny.tile([1, SQ], F32)
            r_dram = dram.tile([1, SQ], F32)
            r8 = tiny.tile([8, P], F32)
            pt8 = psl.tile([P, 8], F32, tag="pt8")
            recip_t = tiny.tile([P, 8], F32)
            pending = []

            def emit_store(qc, eh, ot, i, src=None):
                # src=psum view: fused drain+scale in one DVE op (only
                # once recip_t exists; pending groups copy out first)
                nc.vector.tensor_scalar_mul(ot[:], src if src is not None
                                            else ot[:], recip_t[:, qc:qc + 1])
                eng = nc.sync if i % 2 == 0 else nc.scalar
                eng.dma_start(out.ap()[ts(qc, P), ts(eh, 512)], ot[:])

            def wv_ap(dp, eh):      # [P,512] Wv^T view for d-tile dp
                return wv2[dp // 2][:, (dp % 2) * SQ + eh * 512:
                                    (dp % 2) * SQ + eh * 512 + 512]

            for g, (qc, eh) in enumerate(groups):
                ps = psum.tile([P, 512], F32, tag="mm", name=f"ps_o{qc}_{eh}")
                for dp in range(ND):
                    nc.tensor.matmul(ps[:], ut_ap(dp, qc * P, P),
                                     wv_ap(dp, eh),
                                     start=(dp == 0), stop=(dp == ND - 1))
                ot = pB.tile([P, 512], F32, tag="b", name=f"ot{qc}_{eh}")
                if g < 5:
                    nc.vector.tensor_copy(ot[:], ps[:])
                    pending.append((qc, eh, ot))
                else:
                    emit_store(qc, eh, ot, g, src=ps[:])
                if g == 2:
                    for qh in range(2):
                        pl = psl.tile([1, 512], F32, tag="pl", name=f"pl{qh}")
                        nc.tensor.matmul(pl[:], ones_c[:], lacc[:, ts(qh, 512)],
                                         start=True, stop=True)
                        nc.vector.tensor_copy(l_row[0:1, ts(qh, 512)], pl[:])
                    nc.sync.dma_start(r_dram[:], l_row[:])
                    nc.sync.dma_start(
                        r8[:], r_dram[0, :].rearrange("(a b) -> a b", a=8))
                elif g == 4:
                    nc.tensor.transpose(pt8[:], r8[:], id8[:])
                    nc.vector.reciprocal(recip_t[:], pt8[:])
                    for i, (pqc, peh, pot) in enumerate(pending):
                        emit_store(pqc, peh, pot, i)

    nc.compile()
    _NC_CACHE["nc"] = nc
    return nc


def make_in_maps(query, key, Wq, Wk, Wv):
    query = np.asarray(query, dtype=np.float32)
    key = np.asarray(key, dtype=np.float32)
    wq_np = np.ascontiguousarray(np.asarray(Wq, dtype=np.float32))
    wk_np = np.ascontiguousarray(np.asarray(Wk, dtype=np.float32))
    wvT_np = np.ascontiguousarray(
        np.asarray(Wv, dtype=np.float32).T.astype(ml_dtypes.bfloat16))

    in_maps = []
    for c in range(N_CORES):
        b, h = c // 2, c % 2
        qTn = np.ascontiguousarray(query[b, h * SQ:(h + 1) * SQ, :].T)
        kTn = np.ascontiguousarray(key[b].T)
        krn = np.ascontiguousarray(key[b].astype(ml_dtypes.bfloat16))
        in_maps.append({
            "qT": qTn, "kT": kTn, "kraw": krn,
            "wq": wq_np, "wk": wk_np, "wvT": wvT_np,
        })
    return in_maps


def kernel(query, key, Wq, Wk, Wv):
    in_maps = make_in_maps(query, key, Wq, Wk, Wv)
    nc = _build()
    res = run_bass_kernel_spmd(nc, in_maps, core_ids=list(range(N_CORES)))
    outv = np.empty((B, S, D), dtype=np.float32)
    for c in range(N_CORES):
        b, h = c // 2, c % 2
        outv[b, h * SQ:(h + 1) * SQ, :] = res.results[c]["out"]
    return outv
